# revision 33
# baseline (speedup 1.0000x reference)
"""Multi-head causal attention on 8 TRN2 NeuronCores (Bass/Tile, SPMD).

Layout/sharding (Megatron-style, two SPMD launches, no collectives):
  Launch 1 ("attn"): tensor-parallel over heads. Each of the 8 cores owns
    H/8 = 2 heads. It computes q/k/v projections for those heads over the
    full (B*T, C) input, the causal softmax attention, and writes its
    transposed head output attT_c = [2*64, B*T] = [128, 4096] (fp16).
  Launch 2 ("proj"): data-parallel over rows. Host reshards: core c takes
    rows [c*512, (c+1)*512) of the concatenated head outputs and computes
    y_c = att_rows @ Wp + bp with the full Wp.

v2 design (fp16 datapath, causal-tight windows, merged exp):
  - x / Wq / Wk / Wv / Wp / att all fp16 (halves DMA + SBUF, same PE rate
    as fp32r: 1 cycle/row).  PSUM accumulation stays fp32.
  - Per-batch loop: proj(b) -> attn(b).  Tile's dependency scheduler then
    overlaps attn(b)'s Act/DVE work with proj(b+1)'s PE work.
  - Scores per (b, s-block): window t in [128*ss, T) only (causal-tight),
    chunked at 512 (PSUM bank).  The two heads' score matmuls use disjoint
    PE row groups (partitions 0-63 / 64-127) and run concurrently.
  - Both heads' score chunks land in one [128, 2, 512] PSUM tile so the
    exp is ONE instruction per chunk (3D strided APs) - the Act engine's
    ~300ns/op overhead is the phase-2 co-cost.  The diagonal causal mask
    is accumulated into the score psum by the PE itself (identity
    stationary x (-60000 fp16) mask, start=False), keeping DVE free.
  - AV with V augmented by a ones column (softmax denominator for free),
    accumulated with causally-trimmed windows; denominator reciprocal on
    DVE, partition-broadcast on the (otherwise idle) GpSimd engine.
"""

import os

import numpy as np

try:  # cache compiled executables (incl. embedded NEFFs) across processes
    import jax

    jax.config.update("jax_compilation_cache_dir", "/tmp/jax_cc_cache")
    jax.config.update("jax_persistent_cache_min_compile_time_secs", 0)
    jax.config.update("jax_persistent_cache_min_entry_size_bytes", 0)
except Exception:  # noqa: BLE001 - cache is best-effort
    pass

import concourse.bass as bass
import concourse.bacc as bacc
import concourse.mybir as mybir
import concourse.tile as tile
from concourse import bass_utils
from concourse.bass import ts
from concourse.masks import make_identity

B, T, C, H, D = 4, 1024, 1024, 16, 64
NCORES = 8
HL = H // NCORES          # heads per core (2)
HD = HL * D               # head-dim columns per core (128)
BT = B * T                # 4096 tokens
P = 128                   # partitions
KT = C // P               # contraction subtiles (8)
TSL = 512                 # free-dim tile (PSUM bank = 512 fp32)
NTSL = T // TSL           # t-chunks per sequence (2)
SB = T // P               # s-blocks per sequence (8)
ROWS = BT // NCORES       # output rows per core in launch 2 (512)
VA = D + 1                # per-head V columns + ones column
FP32 = mybir.dt.float32
FP32R = mybir.dt.float32r
FP16 = mybir.dt.float16
AF = mybir.ActivationFunctionType


# ---------------------------------------------------------------- launch 1

FP8 = mybir.dt.float8e4
QK8_SCALE = 256.0        # fp8 weight pre-scale (keeps W out of subnormals)


def _attn_build(nc, qk8=False, mask_pe=False):
    # pre-tiled on host: xT[tt, p, kt, t], w[p, kt, d] - per-partition
    # contiguous DMAs run at full HBM rate
    xT = nc.dram_tensor("xT", [BT // TSL, P, KT, TSL], FP16,
                        kind="ExternalInput").ap()
    wq = nc.dram_tensor("wq", [P, KT, HD], FP16, kind="ExternalInput").ap()
    wk = nc.dram_tensor("wk", [P, KT, HD], FP16, kind="ExternalInput").ap()
    wv = nc.dram_tensor("wv", [P, KT, HD], FP16, kind="ExternalInput").ap()
    mask = nc.dram_tensor("trimask", [P, HL, P], FP32, kind="ExternalInput").ap()
    att = nc.dram_tensor("att", [HD, BT], FP16, kind="ExternalOutput").ap()
    if mask_pe:
        mask16 = nc.dram_tensor("trimask16", [P, P], FP16,
                                kind="ExternalInput").ap()
        return xT, (wq, wk, wv), mask16, att
    if not qk8:
        return xT, (wq, wk, wv), mask, att
    x8 = nc.dram_tensor("x8", [BT // TSL, P, KT, TSL], FP8,
                        kind="ExternalInput").ap()
    w8q = nc.dram_tensor("w8q", [P, KT, HD], FP8, kind="ExternalInput").ap()
    w8k = nc.dram_tensor("w8k", [P, KT, HD], FP8, kind="ExternalInput").ap()
    return xT, (wq, wk, wv), mask, att, x8, (w8q, w8k)


def _attn_body(tc, xT, ws, mask, att, x8=None, w8s=None, mask_pe=False,
               v_dma=False):
    nc = tc.nc
    wq, wk, wv = ws
    qk8 = x8 is not None

    with (
        tc.tile_pool(name="const", bufs=1) as cpool,
        tc.tile_pool(name="xin", bufs=3) as xpool,
        tc.tile_pool(name="qkv", bufs=2) as qkpool,
        tc.tile_pool(name="ptile", bufs=2) as ppool,
        tc.tile_pool(name="small", bufs=3) as spool,
        tc.tile_pool(name="ost", bufs=3) as opool,
        # [128,512]f32 proj/transpose chains (1 bank x 2) + [128,2,512]f32
        # score tiles (2 banks x 2) + [65,512]f32 AV (1 bank x 2) = 8 banks
        tc.tile_pool(name="psA", bufs=2, space="PSUM") as psA,
        tc.tile_pool(name="psS", bufs=2, space="PSUM") as psS,
        tc.tile_pool(name="psV", bufs=2, space="PSUM") as psV,
    ):
        w_sb = {}
        for name in ("wq", "wk", "wv"):
            dt8 = qk8 and name in ("wq", "wk")
            w_sb[name] = cpool.tile([P, KT, HD], FP8 if dt8 else FP16,
                                    tag=f"w_{name}", name=f"w_{name}")
        if qk8:
            w8q, w8k = w8s
            x8_ts = []
        x_t0 = xpool.tile([P, KT, TSL], FP16, tag="x", name="x_t0")
        q4 = KT // 4
        # interleave the first x tile with wq quarter-chunks so the first
        # projection matmuls start as early as possible
        wq_src = w8q if qk8 else wq
        for qi in range(4):
            nc.sync.dma_start(w_sb["wq"][:, qi * q4:(qi + 1) * q4, :],
                              wq_src[:, qi * q4:(qi + 1) * q4, :])
            nc.sync.dma_start(x_t0[:, qi * q4:(qi + 1) * q4, :],
                              xT[0, :, qi * q4:(qi + 1) * q4, :])
        nc.sync.dma_start(w_sb["wk"][:], w8k if qk8 else wk)
        nc.sync.dma_start(w_sb["wv"][:], wv)
        if mask_pe:
            mask_sb = cpool.tile([P, P], FP16, tag="mask")
        else:
            mask_sb = cpool.tile([P, HL, P], FP32, tag="mask")
        nc.sync.dma_start(mask_sb[:], mask)
        ident = cpool.tile([P, P], FP16, tag="ident")
        make_identity(nc, ident[:])

        for b in range(B):
            qt = qkpool.tile([P, T], FP16, tag="qt", name=f"qt{b}")
            kt_sb = qkpool.tile([P, T], FP16, tag="kt", name=f"kt{b}")
            # v16[s, sblock, head, 0:64]=v, [.., 64]=ones (denominator trick)
            v16 = qkpool.tile([P, SB, HL, VA + 1], FP16, tag="v16",
                              name=f"v{b}")
            nc.vector.memset(v16[:, :, :, D], 1.0)

            # ---- projections for this batch's two t-tiles
            for tt2 in range(2):
                tt = 2 * b + tt2
                if tt == 0:
                    x_t = x_t0
                else:
                    x_t = xpool.tile([P, KT, TSL], FP16, tag="x",
                                     name=f"x_t{tt}")
                    half = KT // 2
                    nc.sync.dma_start(x_t[:, :half, :], xT[tt, :, :half, :])
                    nc.sync.dma_start(x_t[:, half:, :], xT[tt, :, half:, :])
                if qk8:
                    x8_t = xpool.tile([P, KT, TSL], FP8, tag="x8",
                                      name=f"x8_t{tt}")
                    half = KT // 2
                    nc.sync.dma_start(x8_t[:, :half, :], x8[tt, :, :half, :])
                    nc.sync.dma_start(x8_t[:, half:, :], x8[tt, :, half:, :])
                for wname, dst in (("wq", qt), ("wk", kt_sb)):
                    ps = psA.tile([P, TSL], FP32, tag="mm")
                    if qk8:
                        # DoubleRow fp8: K=256 per pass, 2x PE throughput
                        for k in range(0, KT, 2):
                            nc.tensor.matmul(
                                ps[:],
                                w_sb[wname][:, k:k + 2, :],
                                x8_t[:, k:k + 2, :],
                                start=(k == 0),
                                stop=(k == KT - 2),
                                perf_mode=mybir.MatmulPerfMode.DoubleRow,
                            )
                        nc.vector.tensor_scalar_mul(dst[:, ts(tt2, TSL)],
                                                    ps[:], 1.0 / QK8_SCALE)
                        continue
                    for k in range(KT):
                        nc.tensor.matmul(
                            ps[:],
                            w_sb[wname][:, k, :],
                            x_t[:, k, :],
                            start=(k == 0),
                            stop=(k == KT - 1),
                        )
                    nc.vector.tensor_copy(dst[:, ts(tt2, TSL)], ps[:])
                ps = psA.tile([P, TSL], FP32, tag="mm")
                for k in range(KT):
                    nc.tensor.matmul(
                        ps[:],
                        w_sb["wv"][:, k, :],
                        x_t[:, k, :],
                        start=(k == 0),
                        stop=(k == KT - 1),
                    )
                vt = spool.tile([P, TSL], FP16, tag="vt")
                nc.vector.tensor_copy(vt[:], ps[:])
                for j in range(TSL // P):
                    g = tt2 * (TSL // P) + j
                    if v_dma:
                        # transpose on the DMA xbar instead of the PE
                        nc.sync.dma_start(v16[:, g, :, 0:D], vt[:, ts(j, P)],
                                          transpose=True)
                        continue
                    pst = psA.tile([P, HL, D], FP16, tag="mm",
                                   name=f"pst{tt}_{j}")
                    nc.tensor.transpose(pst[:, :, :], vt[:, ts(j, P)],
                                        ident[:])
                    nc.vector.tensor_copy(v16[:, g, :, 0:D], pst[:, :, :])

            # ---- scores + exp, causal-tight windows, both heads in one
            # PSUM tile ([128, 2, 512]); heads run on disjoint PE row groups
            p16 = ppool.tile([P, SB, HL, T], FP16, tag="p", name=f"p{b}")
            for ss in range(SB):
                s0 = ss * P
                n_chunk = 2 if ss < 4 else 1
                chunks = []
                for c in range(n_chunk):
                    t0 = s0 + c * TSL
                    w = min(TSL, T - t0)
                    chunks.append((c, t0, w, psS.tile(
                        [P, HL, TSL], FP32, tag="sc", name=f"sc{b}_{ss}_{c}")))
                # h-outer so both chunks of a head reuse the loaded stationary
                for h in range(HL):
                    hp = h * D
                    for c, t0, w, ps in chunks:
                        nc.tensor.matmul(
                            ps[:, h, 0:w],
                            kt_sb[hp:hp + D, s0:s0 + P],
                            qt[hp:hp + D, t0:t0 + w],
                            start=True,
                            stop=True,
                        )
                for c, t0, w, ps in chunks:
                    if c == 0:
                        if mask_pe:
                            # diagonal mask via PE: psum += I.T @ mask
                            for h in range(HL):
                                nc.tensor.matmul(
                                    ps[:, h, 0:P], ident[:], mask_sb[:],
                                    start=False, stop=True,
                                    skip_group_check=True,
                                )
                        else:
                            # diagonal block: additive triangle mask (DVE)
                            nc.vector.tensor_add(ps[:, :, 0:P], ps[:, :, 0:P],
                                                 mask_sb[:])
                    nc.scalar.activation(p16[:, ss, :, t0:t0 + w],
                                         ps[:, :, 0:w], AF.Exp)

            # ---- AV per (head, t-chunk) with causally-trimmed windows;
            # ones-column gives the denominator in psum row D
            for c in range(NTSL):
                ss_hi = min(SB, 4 * (c + 1))
                # both heads' outputs share one [128, TSL] tile so the att
                # store is a single full-partition-width DMA (a [64, N] DMA
                # runs at half bandwidth)
                o16 = opool.tile([P, TSL], FP16, tag="o", name=f"o{b}_{c}")
                for h in range(HL):
                    hp = h * D
                    ps_a = psV.tile([VA, TSL], FP32, tag="av",
                                    name=f"av{b}_{c}_{h}")
                    for ss in range(ss_hi):
                        st = max(0, ss * P - c * TSL)
                        nc.tensor.matmul(
                            ps_a[:, st:],
                            v16[:, ss, h, 0:VA],
                            p16[:, ss, h, c * TSL + st:(c + 1) * TSL],
                            start=(ss == 0),
                            stop=(ss == ss_hi - 1),
                        )
                    den = spool.tile([1, TSL], FP32, tag="den",
                                     name=f"den{b}_{c}_{h}")
                    nc.vector.tensor_copy(den[:], ps_a[D:D + 1, :])
                    rden = spool.tile([1, TSL], FP32, tag="rden",
                                      name=f"rden{b}_{c}_{h}")
                    nc.vector.reciprocal(rden[:], den[:])
                    rb = spool.tile([D, TSL], FP32, tag="rb",
                                    name=f"rb{b}_{c}_{h}")
                    nc.gpsimd.partition_broadcast(rb[:], rden[:])
                    nc.vector.tensor_mul(o16[hp:hp + D, :], ps_a[0:D, :],
                                         rb[:])
                nc.sync.dma_start(
                    att[:, b * T + c * TSL:b * T + (c + 1) * TSL], o16[:])


# ---------------------------------------------------------------- launch 2

def _proj_build(nc):
    attT = nc.dram_tensor("attT", [P, KT, ROWS], FP16, kind="ExternalInput").ap()
    wp = nc.dram_tensor("wp", [P, KT, C], FP16, kind="ExternalInput").ap()
    bp = nc.dram_tensor("bp", [1, C], FP32, kind="ExternalInput").ap()
    y = nc.dram_tensor("y", [ROWS, C], FP32, kind="ExternalOutput").ap()
    return attT, wp, bp, y


def _proj_load(tc, pool, attT, wp, bp):
    """DMA the out-projection inputs; callable before the attn body so the
    3MB of loads prefetch during attention instead of serializing at the
    attn/proj boundary."""
    nc = tc.nc
    a_sb = pool.tile([P, KT, ROWS], FP16, tag="a")
    w_sb = pool.tile([P, KT, C], FP16, tag="w")
    for k in range(KT):
        nc.sync.dma_start(a_sb[:, k, :], attT[:, k, :])
        nc.sync.dma_start(w_sb[:, k, :], wp[:, k, :])
    b1 = pool.tile([1, C], FP32, tag="b1")
    nc.sync.dma_start(b1[:], bp)
    b_sb = pool.tile([P, C], FP32, tag="b")
    nc.gpsimd.partition_broadcast(b_sb[:], b1[:])
    return a_sb, w_sb, b_sb


def _proj_compute(tc, opool, psp, y, a_sb, w_sb, b_sb):
    nc = tc.nc
    for m in range(ROWS // P):
        o_sb = opool.tile([P, C], FP32, tag="o")
        for n in range(C // TSL):
            ps = psp.tile([P, TSL], FP32, tag="mm")
            for k in range(KT):
                nc.tensor.matmul(
                    ps[:],
                    a_sb[:, k, ts(m, P)],
                    w_sb[:, k, ts(n, TSL)],
                    start=(k == 0),
                    stop=(k == KT - 1),
                )
            nc.vector.tensor_add(o_sb[:, ts(n, TSL)], ps[:], b_sb[:, ts(n, TSL)])
        nc.sync.dma_start(y[ts(m, P), :], o_sb[:])


def _proj_body(tc, attT, wp, bp, y):
    nc = tc.nc
    with (
        tc.tile_pool(name="psb", bufs=1) as pool,
        tc.tile_pool(name="po", bufs=3) as opool,
        tc.tile_pool(name="pps", bufs=4, space="PSUM") as psp,
    ):
        tiles = _proj_load(tc, pool, attT, wp, bp)
        _proj_compute(tc, opool, psp, y, *tiles)


# ---------------------------------------------------------------- build/run

_BUILT = {}


def build_nc(which, repeat=1):
    key = (which, repeat)
    if key in _BUILT:
        return _BUILT[key]
    nc = bacc.Bacc(
        "TRN2",
        target_bir_lowering=False,
        debug=False,
        enable_asserts=False,
        num_devices=NCORES,
    )
    if which == "attn":  # PE-side diagonal mask is the default
        aps = _attn_build(nc, mask_pe=True)
        with tile.TileContext(nc) as tc:
            for _ in range(repeat):
                _attn_body(tc, *aps, mask_pe=True)
    elif which == "comb":  # attn+proj in one NEFF (timing: R-delta of the sum)
        aps1 = _attn_build(nc, mask_pe=True)
        attT, wp, bp, y = _proj_build(nc)
        with tile.TileContext(nc) as tc:
            for _ in range(repeat):
                # proj inputs are independent of the attn body (pre-staged by
                # the host reshard), so their DMAs prefetch during attention;
                # only the PSUM pool waits for the attn pools to close.
                with tc.tile_pool(name="psb", bufs=1) as prpool:
                    tiles = _proj_load(tc, prpool, attT, wp, bp)
                    _attn_body(tc, *aps1, mask_pe=True)
                    with (
                        tc.tile_pool(name="po", bufs=3) as propool,
                        tc.tile_pool(name="pps", bufs=4, space="PSUM") as psp,
                    ):
                        _proj_compute(tc, propool, psp, y, *tiles)
    elif which in ("attnD", "combD"):  # DVE-mask variant (A/B reference)
        aps1 = _attn_build(nc)
        aps2 = _proj_build(nc) if which == "combD" else None
        with tile.TileContext(nc) as tc:
            for _ in range(repeat):
                _attn_body(tc, *aps1)
                if aps2 is not None:
                    _proj_body(tc, *aps2)
    elif which in ("attn8", "comb8"):
        aps1 = _attn_build(nc, qk8=True)
        aps2 = _proj_build(nc) if which == "comb8" else None
        with tile.TileContext(nc) as tc:
            for _ in range(repeat):
                _attn_body(tc, aps1[0], aps1[1], aps1[2], aps1[3],
                           x8=aps1[4], w8s=aps1[5])
                if aps2 is not None:
                    _proj_body(tc, *aps2)
    elif which in ("attnM", "combM"):
        aps1 = _attn_build(nc, mask_pe=True)
        aps2 = _proj_build(nc) if which == "combM" else None
        with tile.TileContext(nc) as tc:
            for _ in range(repeat):
                _attn_body(tc, *aps1, mask_pe=True)
                if aps2 is not None:
                    _proj_body(tc, *aps2)
    elif which in ("attnT", "combT"):  # BROKEN numerics (xbar layout) - do not use
        aps1 = _attn_build(nc, mask_pe=True)
        aps2 = _proj_build(nc) if which == "combT" else None
        with tile.TileContext(nc) as tc:
            for _ in range(repeat):
                _attn_body(tc, *aps1, mask_pe=True, v_dma=True)
                if aps2 is not None:
                    _proj_body(tc, *aps2)
    else:
        aps = _proj_build(nc)
        with tile.TileContext(nc) as tc:
            for _ in range(repeat):
                _proj_body(tc, *aps)
    nc.compile()
    _BUILT[key] = nc
    return nc


def host_mask01():
    # additive triangle mask for the 128x128 diagonal: -BIG where s > t
    rows = np.arange(P)[:, None]
    cols = np.arange(P)[None, :]
    return np.where(rows > cols, np.float32(-1.0e30), np.float32(0.0))


def attn_in_maps(x, Wq, Wk, Wv, qk8=False):
    import ml_dtypes

    # xT[tt, p, kt, t] = x[tt*512 + t, kt*128 + p]
    xT4f = np.ascontiguousarray(
        x.reshape(BT // TSL, TSL, KT, P).transpose(0, 3, 2, 1)
    )
    xT4 = xT4f.astype(np.float16)
    if qk8:
        x8 = xT4f.astype(ml_dtypes.float8_e4m3)
    mask2 = np.ascontiguousarray(
        np.broadcast_to(host_mask01()[:, None, :], (P, HL, P))
    )
    scale = np.float32(1.0) / np.sqrt(np.float32(D))
    in_maps = []
    for c in range(NCORES):
        hs = slice(c * HL, (c + 1) * HL)

        def wslice(W, s=1.0, dt=np.float16):
            # [p, kt, hd] = W[kt*128 + p, hd]
            w2 = W[hs].transpose(1, 0, 2).reshape(C, HD) * np.float32(s)
            return np.ascontiguousarray(
                w2.reshape(KT, P, HD).transpose(1, 0, 2)
            ).astype(dt)

        m = {
            "xT": xT4,
            "wq": wslice(Wq, scale),
            "wk": wslice(Wk),
            "wv": wslice(Wv),
            "trimask": mask2,
            "trimask16": np.where(
                np.arange(P)[:, None] > np.arange(P)[None, :],
                np.float16(-60000.0), np.float16(0.0)),
        }
        if qk8:
            m["x8"] = x8
            m["w8q"] = wslice(Wq, scale * QK8_SCALE, ml_dtypes.float8_e4m3)
            m["w8k"] = wslice(Wk, QK8_SCALE, ml_dtypes.float8_e4m3)
        in_maps.append(m)
    return in_maps


def proj_in_maps(att_list, Wp, bp):
    # [p, kt, n] = Wp[kt*128 + p, n]
    wp = np.ascontiguousarray(
        Wp.astype(np.float32, copy=False).reshape(KT, P, C).transpose(1, 0, 2)
    ).astype(np.float16)
    bp2 = np.ascontiguousarray(bp.reshape(1, C).astype(np.float32, copy=False))
    in_maps = []
    for c in range(NCORES):
        attT_c = np.concatenate(
            [a[:, c * ROWS:(c + 1) * ROWS] for a in att_list], axis=0
        )  # [C, ROWS] fp16
        attT_c = np.ascontiguousarray(
            attT_c.reshape(KT, P, ROWS).transpose(1, 0, 2)
        )
        in_maps.append({"attT": attT_c, "wp": wp, "bp": bp2})
    return in_maps


LAST = {}


# ------------------------------------------------------- timing harness
# The axon NTFF profiling hook is unavailable in this container, so HW
# execution time is measured by running the compiled NEFF repeatedly with
# device-resident inputs and taking the slope between two iteration counts
# (removes fixed dispatch/pipeline-fill overhead).

_CALLABLES = {}


def _pjrt_callable(which, repeat=1):
    """jit(shard_map(bass_exec)) over 8 cores, mirroring run_bass_via_pjrt
    but without donation so device input buffers can be reused across calls."""
    if (which, repeat) in _CALLABLES:
        return _CALLABLES[(which, repeat)]
    import jax
    from jax.sharding import Mesh, NamedSharding, PartitionSpec
    from jax.experimental.shard_map import shard_map

    from concourse import bass2jax

    nc = build_nc(which, repeat)
    bass2jax.install_neuronx_cc_hook()
    partition_name = nc.partition_id_tensor.name if nc.partition_id_tensor else None
    in_names, out_names, out_avals, zero_outs = [], [], [], []
    for alloc in nc.m.functions[0].allocations:
        if not isinstance(alloc, mybir.MemoryLocationSet):
            continue
        name = alloc.memorylocations[0].name
        if alloc.kind == "ExternalInput":
            if name != partition_name:
                in_names.append(name)
        elif alloc.kind == "ExternalOutput":
            out_names.append(name)
            shape = tuple(alloc.tensor_shape)
            dtype = mybir.dt.np(alloc.dtype)
            out_avals.append(jax.core.ShapedArray(shape, dtype))
            zero_outs.append(np.zeros(shape, dtype))
    n_params = len(in_names)
    all_in = list(in_names) + list(out_names)
    if partition_name is not None:
        all_in.append(partition_name)

    def _body(*args):
        operands = list(args)
        if partition_name is not None:
            operands.append(bass2jax.partition_id_tensor())
        outs = bass2jax._bass_exec_p.bind(
            *operands,
            out_avals=tuple(out_avals),
            in_names=tuple(all_in),
            out_names=tuple(out_names),
            lowering_input_output_aliases=(),
            sim_require_finite=True,
            sim_require_nnan=True,
            nc=nc,
        )
        return tuple(outs)

    devices = jax.devices()[:NCORES]
    mesh = Mesh(np.asarray(devices), ("core",))
    nspecs = n_params + len(out_names)
    fn = jax.jit(
        shard_map(
            _body,
            mesh=mesh,
            in_specs=(PartitionSpec("core"),) * nspecs,
            out_specs=(PartitionSpec("core"),) * len(out_names),
            check_rep=False,
        ),
        keep_unused=True,
    )
    sharding = NamedSharding(mesh, PartitionSpec("core"))
    res = (fn, in_names, out_names, out_avals, zero_outs, sharding)
    _CALLABLES[(which, repeat)] = res
    return res


def run_fast(which, in_maps):
    """Correctness run through the no-donation callable; returns per-core
    dict like run_bass_kernel_spmd results."""
    import jax

    fn, in_names, out_names, out_avals, zero_outs, sharding = _pjrt_callable(which)
    concat_in = [
        np.concatenate([np.asarray(m[n]) for m in in_maps], axis=0)
        for n in in_names
    ]
    concat_zero = [
        np.zeros((NCORES * z.shape[0], *z.shape[1:]), z.dtype) for z in zero_outs
    ]
    dev = [jax.device_put(a, sharding) for a in concat_in + concat_zero]
    outs = fn(*dev)
    return [
        {
            n: np.asarray(outs[i]).reshape(NCORES, *out_avals[i].shape)[c]
            for i, n in enumerate(out_names)
        }
        for c in range(NCORES)
    ], dev


def _timing_setup(which, r, in_maps):
    import jax

    fn, in_names, out_names, out_avals, zero_outs, sharding = _pjrt_callable(
        which, r
    )
    concat_in = [
        np.concatenate([np.asarray(m[n]) for m in in_maps], axis=0)
        for n in in_names
    ]
    concat_zero = [
        np.zeros((NCORES * z.shape[0], *z.shape[1:]), z.dtype) for z in zero_outs
    ]
    dev = [jax.device_put(a, sharding) for a in concat_in + concat_zero]
    jax.block_until_ready(fn(*dev))  # warm-up / compile
    return fn, dev


def time_hw(which, in_maps, reps=(1, 8), rounds=4, n1=8, n2=40):
    """Per-NEFF-execution HW time (ns).

    Axon per-call latency is large and noisy, so: pipeline n async dispatches
    per measurement (slope over n2-n1 removes pipeline fill), difference the
    slopes of NEFFs with the body repeated reps[1] vs reps[0] times (removes
    per-call overhead), interleave the two variants and take the median over
    rounds (removes drift).
    """
    import time as _time

    import jax

    setups = {r: _timing_setup(which, r, in_maps) for r in reps}

    def run_n(r, n):
        fn, dev = setups[r]
        t0 = _time.perf_counter()
        o = None
        for _ in range(n):
            o = fn(*dev)
        jax.block_until_ready(o)
        return _time.perf_counter() - t0

    for r in reps:
        run_n(r, 3)
    deltas = []
    all_deltas = []
    slopes_log = {r: [] for r in reps}
    for _ in range(rounds):
        slopes = {}
        for r in reps:
            t_a = min(run_n(r, n1) for _ in range(2))
            t_b = min(run_n(r, n2) for _ in range(2))
            slopes[r] = (t_b - t_a) / (n2 - n1) * 1e9
            slopes_log[r].append(slopes[r])
        d = (slopes[reps[1]] - slopes[reps[0]]) / (reps[1] - reps[0])
        all_deltas.append(d)
        # Discard rounds where the small-NEFF leg fell into the deep async
        # pipelining regime (its per-call slope collapses well below the
        # big-NEFF leg's) - the two legs then measure different dispatch
        # modes and the delta is inflated.
        if slopes[reps[0]] >= 0.62 * slopes[reps[1]]:
            deltas.append(d)
    if not deltas:
        deltas = all_deltas
    deltas.sort()
    med = deltas[len(deltas) // 2]
    return med, {r: sorted(v)[len(v) // 2] for r, v in slopes_log.items()}


def time_hw_paired(which, in_maps, reps=(1, 12), n=150):
    """Per-NEFF-execution HW time (ns) via PAIRED BLOCKING calls.

    Async dispatch times through the axon tunnel vary 1.2-2.4ms per call and
    its pipelined throughput saturates at an artifact rate (~200us/repeat),
    so R-delta slopes of async call streams are a dispatch-time lottery.
    Instead: block on every call, alternate the R-small / R-big NEFFs so the
    (~86ms, slowly drifting) sync RTT cancels in the pairwise difference,
    and take the median of (t_big - t_small)/(reps[1]-reps[0]).
    """
    import time as _time

    import jax

    fn1, dev1 = _timing_setup(which, reps[0], in_maps)
    fn2, dev2 = _timing_setup(which, reps[1], in_maps)
    for _ in range(3):  # warm both paths
        jax.block_until_ready(fn1(*dev1))
        jax.block_until_ready(fn2(*dev2))
    diffs = []
    for _ in range(n):
        t0 = _time.perf_counter()
        jax.block_until_ready(fn1(*dev1))
        t1 = _time.perf_counter()
        jax.block_until_ready(fn2(*dev2))
        t2 = _time.perf_counter()
        diffs.append((t2 - t1) - (t1 - t0))
    diffs.sort()
    med = diffs[len(diffs) // 2] / (reps[1] - reps[0]) * 1e9
    return med, {
        "p25": diffs[len(diffs) // 4] / (reps[1] - reps[0]) * 1e9,
        "p75": diffs[3 * len(diffs) // 4] / (reps[1] - reps[0]) * 1e9,
    }


def kernel(x, Wq, Wk, Wv, Wp, bp):
    x = np.asarray(x, dtype=np.float32)
    Wq = np.asarray(Wq, dtype=np.float32)
    Wk = np.asarray(Wk, dtype=np.float32)
    Wv = np.asarray(Wv, dtype=np.float32)
    Wp = np.asarray(Wp, dtype=np.float32)
    bp = np.asarray(bp, dtype=np.float32)

    cores = list(range(NCORES))
    nc1 = build_nc("attn")
    r1 = bass_utils.run_bass_kernel_spmd(nc1, attn_in_maps(x, Wq, Wk, Wv), cores)
    LAST["attn"] = r1
    att_list = [r1.results[c]["att"] for c in range(NCORES)]

    nc2 = build_nc("proj")
    r2 = bass_utils.run_bass_kernel_spmd(nc2, proj_in_maps(att_list, Wp, bp), cores)
    LAST["proj"] = r2
    y = np.concatenate([r2.results[c]["y"] for c in range(NCORES)], axis=0)
    return y.reshape(B, T, C)
